# revision 44
# baseline (speedup 1.0000x reference)
"""LocallyConnected2d (3x3, pad 1) Trainium2 kernel.

Problem: out[b,o,h,w] = sum_{c,k} x_pad[b,c,h+k//3,w+k%3] * W[o,c,h,w,k]
  x: [16, 64, 56, 56] f32, W: [1, 64, 64, 56, 56, 9] f32 -> out [16, 64, 56, 56] f32

Strategy (8 cores, H sharded, 7 rows/core, all bf16 on device):
  The weight is used exactly once per element -> memory bound. bf16 halves
  HBM traffic to ~33 MB/core (~95 us at the 358 GB/s per-core HBM limit);
  output error ~4e-3 vs the 2e-2 gate.

  To amortize per-instruction PE overhead, 7 same-parity output locations
  are batched into ONE matmul per contraction chunk:
    stationary lhsT = x patches [K, 7*16]  (7 locations x 16 batch, l-major)
    moving rhs      = weights   [K, 7*64]  (those locations' weights)
    psum out        = [112, 448], of which only the 7 diagonal [16,64]
                      blocks (l==l') are wanted.
  The off-diagonal compute is free: the weight stream (1 col/cycle bf16,
  each weight element enters the PE exactly once) is the true floor, and
  this shape reaches it with 6 matmuls per 7 locations. Diagonal blocks
  are extracted as 32-partition-aligned copies (ACT quadrant rule)
  alternating between the Vector and Scalar engines; pair copies carry 50%
  junk columns that the host strips.

  x lives in SBUF as (partitions 0-63: plain, 64-127: +58-shifted plain),
  F-parity de-interleaved so a 7-location stride-2 patch is contiguous.
  Contraction chunks per location (K = c x taps, 576 total):
    chunks 0-2: tap pairs {t, t+3}, K=128 (lower reads row h+t%.., upper
                reads the same offset = one padded row lower)
    taps 6,7,8: three K=64 matmuls on row h+2: even-parity groups read the
                plain lower half; odd-parity groups read the shifted upper
                half at row h+1 (same values), so even/odd matmuls sit on
                disjoint PE row groups and overlap.
"""

import numpy as np

B, C, O, H, W = 16, 64, 64, 56, 56
NCORES = 8
HPC = H // NCORES          # 7 output rows per core
XROWS = HPC + 2            # 9 padded-x rows per core
XW = W + 2                 # 58
BLK = XROWS * XW           # 522 x elems per b block per partition
HB = BLK // 2              # halved-F dim (parity de-interleave)
SPR = 2                    # weight slabs per output row
NSLAB = HPC * SPR          # 14 weight slabs per core
WSLAB = W // SPR           # 28 w positions per slab
GRP = 7                    # locations per batched matmul group

K_LO = [0, 1, 2, 6]        # lower-half tap per chunk 0-3 (legacy packing)
K_HI = [3, 4, 5, 7]        # upper-half tap per chunk 0-3

_CACHE = {}


def _bf16(a):
    import ml_dtypes
    return a.astype(ml_dtypes.bfloat16)


def _host_prep(x, weight):
    """Build per-core device input arrays (layout transforms, host-side only)."""
    x = np.ascontiguousarray(x, dtype=np.float32)
    w0 = weight.reshape(O, C, H, W, 9).astype(np.float32, copy=False)

    xpad = np.zeros((B, C, H + 2, W + 2), np.float32)
    xpad[:, :, 1:-1, 1:-1] = x

    xs_list, ws_list = [], []
    for core in range(NCORES):
        h0 = core * HPC
        # x over HBM: the plain block [64, F%2, F//2, b] plus the first 3
        # rows of the +58-shifted copy (so row-0/1 matmuls wait on no
        # SBUF->SBUF derivation); shifted rows 3-7 are derived on-device
        xc = xpad[:, :, h0:h0 + XROWS, :]                     # [B, C, 9, 58]
        plain = np.ascontiguousarray(xc.transpose(1, 2, 3, 0)).reshape(C, BLK, B)
        x4 = plain.reshape(C, HB, 2, B).transpose(0, 2, 1, 3)
        xh = plain[:, 58:58 + 3 * XW, :].reshape(C, 3 * XW // 2, 2, B)
        xh = xh.transpose(0, 2, 1, 3)
        xs_list.append((_bf16(np.ascontiguousarray(x4)),
                        _bf16(np.ascontiguousarray(xh))))

        # weight slabs: S[h, p=(s,c), wp, slot, o] (legacy slot packing)
        wc = w0[:, :, h0:h0 + HPC, :, :]                       # [O, C, 7, 56, 9]
        wt = wc.transpose(2, 1, 3, 4, 0)                       # [7, C, 56, 9, O]
        we = wt[:, :, 0::2]                                    # [7, C, 28, 9, O] even w
        wo = wt[:, :, 1::2]
        S = np.empty((HPC, 128, W // 2, 9, O), np.float32)
        S[:, :64, :, 0:4] = we[:, :, :, K_LO, :]
        S[:, :64, :, 4] = we[:, :, :, 8, :]
        S[:, :64, :, 5:9] = wo[:, :, :, K_LO, :]
        S[:, 64:, :, 0:4] = we[:, :, :, K_HI, :]
        S[:, 64:, :, 4] = wo[:, :, :, 8, :]
        S[:, 64:, :, 5:9] = wo[:, :, :, K_HI, :]
        # split into SPR slabs of 2 w-pair-halves (wph), 7 w-pairs each
        Sr = S.reshape(HPC, 128, SPR, 2, GRP, 9, O).transpose(0, 2, 3, 1, 4, 5, 6)
        Sr = np.ascontiguousarray(Sr)      # [HPC, SPR, wph, 128, l, slot, O]
        Sr = Sr.reshape(NSLAB, 2, 128, GRP, 9, O)
        # per-(group, chunk) contiguous [448] blocks, one half-slab per DMA:
        # b0-2 = even chunks 0-2, b3-5 = odd chunks 0-2,
        # b6/b7/b8 = taps 6/7/8 with (lower: even w, upper: odd w)
        T = np.empty((NSLAB, 2, 128, 9, GRP, O), np.float32)
        T[:, :, :, 0:3] = Sr[:, :, :, :, 0:3].transpose(0, 1, 2, 4, 3, 5)
        T[:, :, :, 3:6] = Sr[:, :, :, :, 5:8].transpose(0, 1, 2, 4, 3, 5)
        T[:, :, 0:64, 6] = Sr[:, :, 0:64, :, 3]    # tap6 even
        T[:, :, 64:, 6] = Sr[:, :, 0:64, :, 8]     # tap6 odd
        T[:, :, 0:64, 7] = Sr[:, :, 64:, :, 3]     # tap7 even
        T[:, :, 64:, 7] = Sr[:, :, 64:, :, 8]      # tap7 odd
        T[:, :, :, 8] = Sr[:, :, :, :, 4]          # tap8 (even | odd)
        ws_list.append(_bf16(np.ascontiguousarray(T).reshape(
            NSLAB, 2, 128, 9, GRP * O)))
    return xs_list, ws_list


def _build_program(mode="full"):
    import concourse.mybir as mybir
    import concourse.tile as tile
    from concourse import bacc

    f32 = mybir.dt.float32
    bf16 = mybir.dt.bfloat16
    nc = bacc.Bacc("TRN2", target_bir_lowering=False, debug=False,
                   num_devices=NCORES)
    xs = nc.dram_tensor("xs", [C, 2, HB, B], bf16, kind="ExternalInput")
    xh = nc.dram_tensor("xh", [C, 2, 3 * XW // 2, B], bf16,
                        kind="ExternalInput")
    ws = nc.dram_tensor("ws", [NSLAB, 2, 128, 9, GRP * O], bf16,
                        kind="ExternalInput")
    # out row: partition l*16+b (l = location lane), free (group, pair-col);
    # host strips the 50%-junk pair columns. bf16 halves output traffic.
    out = nc.dram_tensor("out", [HPC, GRP * B, 8 * 2 * O], bf16,
                         kind="ExternalOutput")

    with tile.TileContext(nc) as tc:
        with tc.tile_pool(name="xp", bufs=1) as xpool, \
             tc.tile_pool(name="wp", bufs=6) as wpool, \
             tc.tile_pool(name="op", bufs=2) as opool, \
             tc.tile_pool(name="pp", bufs=8, space="PSUM") as ppool:

            xt = xpool.tile([128, 2, HB, B], bf16, name="xt")
            nc.scalar.dma_start(xt[0:64], xs[:])
            # shifted rows 0-2 straight from HBM (first matmuls wait on no
            # derivation); rows 3-7 derived per-row on the SWDGE queue
            # (sh58[F] = plain[F+58], parity preserved: +29 on halved-F),
            # each needed only when the PE reaches output row r-1 (~2 us
            # per row behind the weight stream - ample slack)
            nc.scalar.dma_start(xt[64:128, :, 0:3 * XW // 2, :], xh[:])
            for r in range(3, XROWS - 1):
                nc.gpsimd.dma_start(
                    xt[64:128, :, 29 * r:29 * r + 29, :],
                    xt[0:64, :, 29 * r + 29:29 * r + 58, :])

            def xap(lo, hi, F0):
                # [hi-lo, 7, B] x patch: 7 locations starting at offset F0
                # with stride 2 (one step of the halved-F dim in the F0%2
                # parity plane); (7, B) is contiguous -> one 112-wide free
                # dim
                return xt[lo:hi, F0 % 2, F0 // 2:F0 // 2 + GRP, :]

            def diag_copy(ot, ps, gi, ncopy):
                # extract the diagonal as 32-aligned blocks: three [32, 128]
                # pair copies (50% junk cols) + one [16, 64] at base 96
                oc0 = gi * 2 * O
                for a in range(3):
                    dst = ot[32 * a:32 * a + 32, oc0:oc0 + 2 * O]
                    src = ps[32 * a:32 * a + 32, 2 * a * O:2 * a * O + 2 * O]
                    if ncopy % 2 == 0:
                        nc.vector.tensor_copy(dst, src)
                    else:
                        nc.scalar.copy(dst, src)
                    ncopy += 1
                dst = ot[96:112, oc0:oc0 + O]
                src = ps[96:112, 6 * O:7 * O]
                if ncopy % 2 == 0:
                    nc.vector.tensor_copy(dst, src)
                else:
                    nc.scalar.copy(dst, src)
                return ncopy + 1

            ncopy = 0
            wt0 = None
            if mode == "pe":
                wt0 = wpool.tile([128, 9, GRP * O], bf16, name="wt")
                nc.sync.dma_start(wt0[:], ws[0, 0])
            for h in range(HPC):
                ot = opool.tile([GRP * B, 8 * 2 * O], bf16, name="ot")
                if mode == "dma":
                    nc.vector.memset(ot[:], 0.0)
                for sub in range(SPR):
                    slab = h * SPR + sub
                    for wph in range(2):
                        if mode == "pe":
                            wt = wt0
                        else:
                            wt = wpool.tile([128, 9, GRP * O], bf16,
                                            name="wt")
                            nc.sync.dma_start(wt[:], ws[slab, wph])
                        if mode == "dma":
                            continue
                        we0 = sub * WSLAB + 14 * wph      # first even w
                        pse = ppool.tile([GRP * B, GRP * O], f32, name="ps")
                        pso = ppool.tile([GRP * B, GRP * O], f32, name="ps")
                        # chunks 0-2: tap pairs {t, t+3}, K=128
                        for t in range(3):
                            nc.tensor.matmul(
                                pse[:, :], xap(0, 128, h * XW + we0 + t),
                                wt[:, t, :], start=(t == 0), stop=False)
                        for t in range(3):
                            nc.tensor.matmul(
                                pso[:, :], xap(0, 128, h * XW + we0 + 1 + t),
                                wt[:, 3 + t, :], start=(t == 0), stop=False)
                        # taps 6-8: K=64; even groups read plain (lower,
                        # row h+2), odd groups read the +58-shifted upper
                        # half at row h+1 (same values) -> disjoint PE row
                        # groups, so interleaved even/odd matmuls overlap
                        Fe = (h + 2) * XW + we0
                        Fo = (h + 1) * XW + we0 + 1
                        for t in range(3):
                            nc.tensor.matmul(
                                pse[:, :], xap(0, 64, Fe + t),
                                wt[0:64, 6 + t, :],
                                start=False, stop=(t == 2))
                            nc.tensor.matmul(
                                pso[:, :], xap(64, 128, Fo + t),
                                wt[64:128, 6 + t, :],
                                start=False, stop=(t == 2))
                        gi = sub * 4 + wph * 2
                        ncopy = diag_copy(ot, pse, gi, ncopy)
                        ncopy = diag_copy(ot, pso, gi + 1, ncopy)
                        # drain these two groups as soon as they are ready
                        # (smaller final quantum shortens the end-of-kernel
                        # tail; 512 B/partition is still at line rate)
                        c0 = gi * 2 * O
                        nc.scalar.dma_start(out[h, :, c0:c0 + 2 * 2 * O],
                                            ot[:, c0:c0 + 2 * 2 * O])
    nc.compile()
    return nc


def _get_program(mode="full"):
    key = ("nc", mode)
    if key not in _CACHE:
        _CACHE[key] = _build_program(mode)
    return _CACHE[key]


def run(x, weight, trace=False, mode="full"):
    from concourse.bass_utils import run_bass_kernel_spmd

    nc = _get_program(mode)
    xs_list, ws_list = _host_prep(np.asarray(x), np.asarray(weight))
    in_maps = [{"xs": xs_list[i][0], "xh": xs_list[i][1],
                "ws": ws_list[i]} for i in range(NCORES)]
    res = run_bass_kernel_spmd(nc, in_maps, core_ids=list(range(NCORES)),
                               trace=trace)
    full = np.empty((B, O, H, W), np.float32)
    for i in range(NCORES):
        oc = np.asarray(res.results[i]["out"])       # [HPC, GRP*B, 8*2*O]
        for gi in range(8):
            sub, g = divmod(gi, 4)
            ws0 = sub * WSLAB + 14 * (g // 2) + (g % 2)
            for l in range(GRP):
                a, r = divmod(l, 2)
                blk = oc[:, 32 * a + 16 * r:32 * a + 16 * r + B,
                         (2 * gi + r) * O:(2 * gi + r + 1) * O]
                # [h, b, o] -> [b, o, h]
                full[:, :, i * HPC:(i + 1) * HPC, ws0 + 2 * l] = \
                    blk.transpose(1, 2, 0)
    return full, res


def kernel(x, weight):
    out, _ = run(x, weight, trace=False)
    return out
